# revision 3
# baseline (speedup 1.0000x reference)
"""Multi-head attention block on 8 Trainium2 NeuronCores.

Problem: B=8, N=1024, E=768, H=12, D=64 attention (QKV proj -> softmax(QK^T/8)V
-> output proj), fp32 I/O.

Sharding: data parallel over batch — core b computes batch element b entirely
locally; no collectives. Host shards/stacks.

Per-core kernel (all matmuls in fp32r — hardware TF32-like mode, 1 cyc/row):
  phase 0: DMA x -> PE-transpose -> xT [E, N] (feature-major)
  phase 1: qT/kT = W_qkv.T-major projections [2 heads x 64, N] per tile;
           V natural [N, 65*12] with a ones column per head (65th column)
  phase 2: per head: S^T[k,q] = K^T.T Q^T; exp on ACT (scale=1/8, psum->sbuf);
           U_aug[65, q] = [V | 1].T expS  (row 64 = softmax denominator Z);
           invZ = 1/Z; broadcast via K=1 matmul; attnT_h = U * bcast(invZ)
  phase 3: out[t, e] = sum_h attnT_h.T W_proj_h + b_proj
"""
import numpy as np

B, N, E, H, D = 8, 1024, 768, 12, 64
SCALE = D ** -0.5
NT = N // 128   # token chunks (8)
NE = E // 128   # embed chunks (6)
NQ = N // 512   # moving-dim tiles (2)


def _build():
    import concourse.bacc as bacc
    import concourse.mybir as mybir
    import concourse.tile as tile
    from concourse.masks import make_identity

    F32 = mybir.dt.float32
    F32R = mybir.dt.float32r
    EXP = mybir.ActivationFunctionType.Exp

    nc = bacc.Bacc("TRN2", target_bir_lowering=False)
    x_d = nc.declare_dram_parameter("x", [N, E], F32, isOutput=False)
    wqkv_d = nc.declare_dram_parameter("W_qkv", [E, 3 * E], F32, isOutput=False)
    bqkv_d = nc.declare_dram_parameter("b_qkv", [3 * E], F32, isOutput=False)
    wproj_d = nc.declare_dram_parameter("W_proj", [E, E], F32, isOutput=False)
    bproj_d = nc.declare_dram_parameter("b_proj", [E], F32, isOutput=False)
    out_d = nc.declare_dram_parameter("out", [N, E], F32, isOutput=True)

    with tile.TileContext(nc) as tc:
        with (
            tc.tile_pool(name="const", bufs=1) as cp,
            tc.tile_pool(name="qkv", bufs=1) as qp,
            tc.tile_pool(name="psum", bufs=1, space="PSUM") as ps,
        ):
            # ---- constants ----
            ident = cp.tile([128, 128], F32)
            make_identity(nc, ident)
            ones1f = cp.tile([1, 128], F32)
            nc.vector.memset(ones1f, 1.0)
            ones1 = cp.tile([1, 128], F32R)
            nc.vector.tensor_copy(ones1, ones1f)
            ones65f = cp.tile([65, 64], F32)
            nc.vector.memset(ones65f, 1.0)
            ones65 = cp.tile([65, 64], F32R)
            nc.vector.tensor_copy(ones65, ones65f)
            # per-feature-chunk bias columns for Q and K (fc 0..11)
            bq_cols = []
            for fc in range(12):
                bq_c = cp.tile([128, 1], F32, name=f"bq_{fc}", tag=f"bq_{fc}")
                nc.sync.dma_start(
                    out=bq_c,
                    in_=bqkv_d[fc * 128:(fc + 1) * 128].rearrange("(p o) -> p o", o=1),
                )
                bq_cols.append(bq_c)
            # V-bias broadcast [128, 768]: ones1.T @ b_qkv[1536:2304]
            bv_rowf = cp.tile([1, E], F32)
            nc.sync.dma_start(
                out=bv_rowf, in_=bqkv_d[2 * E:3 * E].rearrange("(o f) -> o f", o=1))
            bv_row = cp.tile([1, E], F32R)
            nc.vector.tensor_copy(bv_row, bv_rowf)
            bv_bc = cp.tile([128, E], F32)
            for nf, (f0, fw) in enumerate([(0, 512), (512, 256)]):
                pbv = ps.tile([128, 512], F32, name=f"pbv{nf}", tag="mm")
                nc.tensor.matmul(pbv[:, :fw], ones1, bv_row[:, f0:f0 + fw],
                                 start=True, stop=True)
                nc.vector.tensor_copy(bv_bc[:, f0:f0 + fw], pbv[:, :fw])
            # proj-bias broadcast [128, 768]
            bp_rowf = cp.tile([1, E], F32)
            nc.sync.dma_start(
                out=bp_rowf, in_=bproj_d[:].rearrange("(o f) -> o f", o=1))
            bp_row = cp.tile([1, E], F32R)
            nc.vector.tensor_copy(bp_row, bp_rowf)
            bp_bc = cp.tile([128, E], F32)
            for nf, (f0, fw) in enumerate([(0, 512), (512, 256)]):
                pbp = ps.tile([128, 512], F32, name=f"pbp{nf}", tag="mm")
                nc.tensor.matmul(pbp[:, :fw], ones1, bp_row[:, f0:f0 + fw],
                                 start=True, stop=True)
                nc.vector.tensor_copy(bp_bc[:, f0:f0 + fw], pbp[:, :fw])

            # ---- long-lived QKV-projection outputs ----
            # qT[c]/kT[c]: [128, N] feature-major, rows = head 2c (0:64) and
            # head 2c+1 (64:128) d-dims.  vS[i]: [128, 780] token-major V with
            # a ones column at 65*h+64.
            qT = [qp.tile([128, N], F32R, name=f"qT{c}", tag=f"qT{c}")
                  for c in range(6)]
            kT = [qp.tile([128, N], F32R, name=f"kT{c}", tag=f"kT{c}")
                  for c in range(6)]
            vS = [qp.tile([128, 65 * H], F32R, name=f"vS{i}", tag=f"vS{i}")
                  for i in range(NT)]

            with tc.tile_pool(name="xw", bufs=1) as xp:
                # ---- phase 0: load x, build xT (feature-major, f32r) ----
                wq_sb = [xp.tile([128, 3 * E], F32R, name=f"wq{j}", tag=f"wq{j}")
                         for j in range(NE)]
                for j in range(NE):
                    nc.gpsimd.dma_start(
                        out=wq_sb[j], in_=wqkv_d[j * 128:(j + 1) * 128, :])
                xT = [xp.tile([128, N], F32R, name=f"xT{j}", tag=f"xT{j}")
                      for j in range(NE)]
                for i in range(NT):
                    xt_i = xp.tile([128, E], F32, name=f"xt{i}", tag="xt", bufs=3)
                    nc.sync.dma_start(out=xt_i, in_=x_d[i * 128:(i + 1) * 128, :])
                    for j in range(NE):
                        pt = ps.tile([128, 128], F32, name=f"pt{i}_{j}", tag="mm")
                        nc.tensor.transpose(
                            pt, xt_i[:, j * 128:(j + 1) * 128], ident)
                        nc.vector.tensor_copy(
                            xT[j][:, i * 128:(i + 1) * 128], pt)

                # ---- phase 1: QKV projections ----
                # Q^T / K^T feature-major: lhsT = W chunk cols, rhs = xT
                for c in range(12):  # 0..5 -> qT, 6..11 -> kT
                    dst = qT[c] if c < 6 else kT[c - 6]
                    wcol0 = c * 128  # W_qkv columns [wcol0, wcol0+128)
                    for q in range(NQ):
                        pq = ps.tile([128, 512], F32, name=f"pq{c}_{q}", tag="mm")
                        for j in range(NE):
                            nc.tensor.matmul(
                                pq,
                                wq_sb[j][:, wcol0:wcol0 + 128],
                                xT[j][:, q * 512:(q + 1) * 512],
                                start=(j == 0), stop=(j == NE - 1))
                        nc.vector.tensor_scalar_add(
                            dst[:, q * 512:(q + 1) * 512], pq, bq_cols[c])
                # V token-major: lhsT = xT chunk, rhs = W_qkv V columns
                vcol0 = 2 * E
                onesH = xp.tile([128, H], F32)
                nc.vector.memset(onesH, 1.0)
                for i in range(NT):
                    # ones columns (65*h + 64)
                    nc.vector.tensor_copy(
                        vS[i].rearrange("p (h c) -> p h c", c=65)[:, :, 64:65],
                        onesH.rearrange("p (h o) -> p h o", o=1))
                    for nf, (f0, fw) in enumerate([(0, 512), (512, 256)]):
                        pv = ps.tile([128, 512], F32, name=f"pv{i}_{nf}", tag="mm")
                        for j in range(NE):
                            nc.tensor.matmul(
                                pv[:, :fw],
                                xT[j][:, i * 128:(i + 1) * 128],
                                wq_sb[j][:, vcol0 + f0:vcol0 + f0 + fw],
                                start=(j == 0), stop=(j == NE - 1))
                        nh = fw // D
                        h0 = f0 // D
                        nc.vector.tensor_add(
                            vS[i].rearrange("p (h c) -> p h c", c=65)
                                [:, h0:h0 + nh, 0:64],
                            pv[:, :fw].rearrange("p (h d) -> p h d", d=D),
                            bv_bc[:, f0:f0 + fw].rearrange(
                                "p (h d) -> p h d", d=D))

            # ---- phase 2: attention per head ----
            with tc.tile_pool(name="attn", bufs=1) as ap:
                attnT = [ap.tile([64, N], F32R, name=f"attnT{h}", tag=f"attnT{h}")
                         for h in range(H)]
                with tc.tile_pool(name="exp", bufs=1) as ep:
                    for h in range(H):
                        c, r0 = h // 2, (h % 2) * 64
                        expS = [
                            ep.tile([128, N], F32R, name=f"expS{h}_{kc}",
                                    tag="expS", bufs=10)
                            for kc in range(NT)]
                        for kc in range(NT):
                            for q in range(NQ):
                                pss = ps.tile([128, 512], F32,
                                              name=f"ps{h}_{kc}_{q}", tag="s",
                                              bufs=2)
                                nc.tensor.matmul(
                                    pss,
                                    kT[c][r0:r0 + 64, kc * 128:(kc + 1) * 128],
                                    qT[c][r0:r0 + 64, q * 512:(q + 1) * 512],
                                    start=True, stop=True)
                                nc.scalar.activation(
                                    expS[kc][:, q * 512:(q + 1) * 512], pss,
                                    EXP, scale=float(SCALE))
                        for q in range(NQ):
                            pu = ps.tile([65, 512], F32, name=f"pu{h}_{q}",
                                         tag="u", bufs=2)
                            for kc in range(NT):
                                nc.tensor.matmul(
                                    pu,
                                    vS[kc][:, h * 65:h * 65 + 65],
                                    expS[kc][:, q * 512:(q + 1) * 512],
                                    start=(kc == 0), stop=(kc == NT - 1))
                            rz = ep.tile([65, 512], F32, name=f"rz{h}_{q}",
                                         tag="rz", bufs=2)
                            nc.vector.reciprocal(rz[64:65, :], pu[64:65, :])
                            rzr = ep.tile([65, 512], F32R, name=f"rzr{h}_{q}",
                                          tag="rzr", bufs=2)
                            nc.vector.tensor_copy(rzr[64:65, :], rz[64:65, :])
                            pb = ps.tile([64, 512], F32, name=f"pb{h}_{q}",
                                         tag="b", bufs=1)
                            nc.tensor.matmul(
                                pb, ones65[64:65, :], rzr[64:65, :],
                                start=True, stop=True)
                            pbs = ep.tile([64, 512], F32, name=f"pbs{h}_{q}",
                                          tag="pbs", bufs=2)
                            nc.vector.tensor_copy(pbs, pb)
                            nc.vector.tensor_mul(
                                attnT[h][:, q * 512:(q + 1) * 512],
                                pu[0:64, :], pbs)

                # ---- phase 3: output projection ----
                with tc.tile_pool(name="proj", bufs=1) as pp:
                    wp_sb = [pp.tile([64, E], F32R, name=f"wp{h}", tag=f"wp{h}")
                             for h in range(H)]
                    for h in range(H):
                        nc.gpsimd.dma_start(
                            out=wp_sb[h], in_=wproj_d[h * 64:(h + 1) * 64, :])
                    for i in range(NT):
                        o_sb = pp.tile([128, E], F32, name=f"o{i}", tag="o",
                                       bufs=3)
                        for nf, (f0, fw) in enumerate([(0, 512), (512, 256)]):
                            po = ps.tile([128, 512], F32, name=f"po{i}_{nf}",
                                         tag="mm")
                            for h in range(H):
                                nc.tensor.matmul(
                                    po[:, :fw],
                                    attnT[h][:, i * 128:(i + 1) * 128],
                                    wp_sb[h][:, f0:f0 + fw],
                                    start=(h == 0), stop=(h == H - 1))
                            nc.vector.tensor_add(
                                o_sb[:, f0:f0 + fw], po[:, :fw],
                                bp_bc[:, f0:f0 + fw])
                        nc.sync.dma_start(
                            out=out_d[i * 128:(i + 1) * 128, :], in_=o_sb)
    nc.compile()
    return nc


_NC_CACHE = None


def kernel(x, W_qkv, b_qkv, W_proj, b_proj):
    from concourse.bass_utils import run_bass_kernel_spmd

    global _NC_CACHE
    if _NC_CACHE is None:
        _NC_CACHE = _build()
    nc = _NC_CACHE

    x = np.ascontiguousarray(np.asarray(x, dtype=np.float32))
    W_qkv = np.ascontiguousarray(np.asarray(W_qkv, dtype=np.float32))
    b_qkv = np.ascontiguousarray(np.asarray(b_qkv, dtype=np.float32))
    W_proj = np.ascontiguousarray(np.asarray(W_proj, dtype=np.float32))
    b_proj = np.ascontiguousarray(np.asarray(b_proj, dtype=np.float32))

    in_maps = [
        {"x": x[b], "W_qkv": W_qkv, "b_qkv": b_qkv,
         "W_proj": W_proj, "b_proj": b_proj}
        for b in range(B)
    ]
    res = run_bass_kernel_spmd(nc, in_maps, core_ids=list(range(B)))
    return np.stack([np.asarray(res.results[b]["out"]) for b in range(B)])


# revision 12
# speedup vs baseline: 16705.3222x; 16705.3222x over previous
"""Multi-head attention block on 8 Trainium2 NeuronCores.

Problem: B=8, N=1024, E=768, H=12, D=64 attention (QKV proj -> softmax(QK^T/8)V
-> output proj), fp32 I/O.

Sharding: data parallel over batch — core b computes batch element b entirely
locally; no collectives. Host shards/stacks.

Per-core kernel (matmuls in fp32r — hardware TF32-like mode, 1 cyc/row):
  phase 0: DMA x -> PE-transpose (batched 4 per psum tile) -> xT [E, N]
  phase 1: V natural [N, 65*12] with a ones column per head (col 65h+64),
           then qT/kT pairs [128, N]: rows (h%2)*64 hold head h's d-dims.
           Q/K bias via per-partition scalar add at psum evac; V/proj bias
           via ones-outer-product broadcast tiles added at evac.
  phase 2: per head: S^T[k,q] = K^T.T Q^T (two 512-wide matmuls into one
           [128,1024] psum); exp on ACT (scale=1/8, one 1024-wide op);
           U_aug[65, q] = [V | 1].T expS (row 64 = softmax denominator Z);
           invZ = 1/Z (f32r); K=1 matmul broadcasts invZ to 64 rows;
           attnT pair tile rows 0:64 (even head, DVE) / 64:128 (odd head,
           DVE -> staging -> partition-shift SBUF DMA)
  phase 3: out[t, e] = sum_c attnT[c].T W_proj[c] + b_proj (128-contraction)
"""
import numpy as np

B, N, E, H, D = 8, 1024, 768, 12, 64
SCALE = D ** -0.5
NT = N // 128   # token chunks (8)
NE = E // 128   # embed chunks (6)
NQ = N // 512   # moving-dim tiles (2)
NFS = [(0, 512), (512, 256)]  # free-dim split of E for matmuls


def _build():
    import concourse.bacc as bacc
    import concourse.mybir as mybir
    import concourse.tile as tile
    from concourse.masks import make_identity

    F32 = mybir.dt.float32
    F32R = mybir.dt.float32r
    EXP = mybir.ActivationFunctionType.Exp

    nc = bacc.Bacc("TRN2", target_bir_lowering=False)
    x_d = nc.declare_dram_parameter("x", [N, E], F32, isOutput=False)
    wqkv_d = nc.declare_dram_parameter("W_qkv", [E, 3 * E], F32, isOutput=False)
    bqkv_d = nc.declare_dram_parameter("b_qkv", [3 * E], F32, isOutput=False)
    wproj_d = nc.declare_dram_parameter("W_proj", [E, E], F32, isOutput=False)
    bproj_d = nc.declare_dram_parameter("b_proj", [E], F32, isOutput=False)
    out_d = nc.declare_dram_parameter("out", [N, E], F32, isOutput=True)

    with tile.TileContext(nc) as tc:
        with (
            tc.tile_pool(name="const", bufs=1) as cp,
            tc.tile_pool(name="qkv", bufs=1) as qp,
            tc.tile_pool(name="psum", bufs=1, space="PSUM") as ps,
        ):
            # ---- constants ----
            identf = cp.tile([128, 128], F32)
            make_identity(nc, identf)
            ident = cp.tile([128, 128], F32R)
            nc.vector.tensor_copy(ident, identf)
            ones1f = cp.tile([1, 128], F32)
            nc.vector.memset(ones1f, 1.0)
            ones1 = cp.tile([1, 128], F32R)
            nc.vector.tensor_copy(ones1, ones1f)
            ones65f = cp.tile([65, 64], F32)
            nc.vector.memset(ones65f, 1.0)
            ones65 = cp.tile([65, 64], F32R)
            nc.vector.tensor_copy(ones65, ones65f)
            bq_cols = [cp.tile([128, 1], F32, name=f"bq_{fc}", tag=f"bq_{fc}")
                       for fc in range(12)]

            # ---- long-lived attention-layout tensors ----
            qT = [qp.tile([128, N], F32R, name=f"qT{c}", tag=f"qT{c}")
                  for c in range(6)]
            kT = [qp.tile([128, N], F32R, name=f"kT{c}", tag=f"kT{c}")
                  for c in range(6)]
            vS = [qp.tile([128, 65 * H], F32R, name=f"vS{i}", tag=f"vS{i}")
                  for i in range(NT)]
            attnT = [qp.tile([128, N], F32R, name=f"attnT{p}", tag=f"attnT{p}")
                     for p in range(6)]

            with tc.tile_pool(name="xw", bufs=1) as xp:
                # ---- phase 0: load x (HWDGE, first), transpose batched ----
                xT = [xp.tile([128, N], F32R, name=f"xT{j}", tag=f"xT{j}")
                      for j in range(NE)]
                xts = {}
                for i in range(NT):
                    xt_i = xp.tile([128, E], F32R, name=f"xt{i}", tag="xt",
                                   bufs=4)
                    nc.gpsimd.dma_start(
                        out=xt_i, in_=x_d[i * 128:(i + 1) * 128, :])
                    xts[i] = xt_i
                # Q/K bias columns, queued on HWDGE after the x tiles
                for fc in range(12):
                    nc.sync.dma_start(
                        out=bq_cols[fc],
                        in_=bqkv_d[fc * 128:(fc + 1) * 128].rearrange(
                            "(p o) -> p o", o=1))
                # weights: V-bias row first, then V columns, then QK columns
                # SWDGE queue order gives x transfer priority over weights
                bv_row = xp.tile([1, E], F32R)
                nc.gpsimd.dma_start(
                    out=bv_row,
                    in_=bqkv_d[2 * E:3 * E].rearrange("(o f) -> o f", o=1))
                wqv = [xp.tile([128, E], F32R, name=f"wqv{j}", tag=f"wqv{j}")
                       for j in range(NE)]
                for j in range(NE):
                    nc.gpsimd.dma_start(
                        out=wqv[j], in_=wqkv_d[j * 128:(j + 1) * 128, 2 * E:])
                wqk = [xp.tile([128, 2 * E], F32R, name=f"wqk{j}", tag=f"wqk{j}")
                       for j in range(NE)]
                for j in range(NE):
                    nc.gpsimd.dma_start(
                        out=wqk[j], in_=wqkv_d[j * 128:(j + 1) * 128, 0:2 * E])
                for ig in range(2):
                    for j in range(NE):
                        pt = ps.tile([128, 512], F32R, name=f"pt{ig}_{j}",
                                     tag="s2", bufs=2)
                        for ii in range(4):
                            i = ig * 4 + ii
                            nc.tensor.transpose(
                                pt[:, ii * 128:(ii + 1) * 128],
                                xts[i][:, j * 128:(j + 1) * 128], ident)
                        nc.vector.tensor_copy(
                            xT[j][:, ig * 512:(ig + 1) * 512], pt)

                # ---- phase 1a: V token-major with ones cols + bias ----
                onesH = xp.tile([128, H], F32)
                nc.vector.memset(onesH, 1.0)
                bv_bc = xp.tile([128, E], F32)
                for nf, (f0, fw) in enumerate(NFS):
                    pbv = ps.tile([128, 512], F32, name=f"pbv{nf}", tag="mm",
                                  bufs=2)
                    nc.tensor.matmul(pbv[:, :fw], ones1, bv_row[:, f0:f0 + fw],
                                     start=True, stop=True)
                    nc.vector.tensor_copy(bv_bc[:, f0:f0 + fw], pbv[:, :fw])
                for i in range(NT):
                    nc.vector.tensor_copy(
                        vS[i].rearrange("p (h c) -> p h c", c=65)[:, :, 64:65],
                        onesH.rearrange("p (h o) -> p h o", o=1))
                    for nf, (f0, fw) in enumerate(NFS):
                        pv = ps.tile([128, 512], F32, name=f"pv{i}_{nf}",
                                     tag=("mm" if (i * 2 + nf) % 2 else "u"),
                                     bufs=2)
                        for j in range(NE):
                            nc.tensor.matmul(
                                pv[:, :fw],
                                xT[j][:, i * 128:(i + 1) * 128],
                                wqv[j][:, f0:f0 + fw],
                                start=(j == 0), stop=(j == NE - 1))
                        nh, h0 = fw // D, f0 // D
                        nc.vector.tensor_add(
                            vS[i].rearrange("p (h c) -> p h c", c=65)
                                [:, h0:h0 + nh, 0:64],
                            pv[:, :fw].rearrange("p (h d) -> p h d", d=D),
                            bv_bc[:, f0:f0 + fw].rearrange(
                                "p (h d) -> p h d", d=D))

                # ---- phase 1b: Q^T / K^T feature-major pairs + bias ----
                for c in range(12):  # 0..5 -> qT, 6..11 -> kT
                    dst = qT[c] if c < 6 else kT[c - 6]
                    wcol0 = c * 128
                    for q in range(NQ):
                        pq = ps.tile([128, 512], F32, name=f"pq{c}_{q}",
                                     tag="mm", bufs=2)
                        for j in range(NE):
                            nc.tensor.matmul(
                                pq,
                                wqk[j][:, wcol0:wcol0 + 128],
                                xT[j][:, q * 512:(q + 1) * 512],
                                start=(j == 0), stop=(j == NE - 1))
                        nc.vector.tensor_scalar_add(
                            dst[:, q * 512:(q + 1) * 512], pq, bq_cols[c])

            # ---- phases 2+3: proj pool first so W_proj loads overlap
            # attention; exp pool released before proj matmuls need space ----
            with tc.tile_pool(name="proj", bufs=1) as pp:
                wp_sb = [pp.tile([128, E], F32R, name=f"wp{c}", tag=f"wp{c}")
                         for c in range(6)]
                for c in range(6):
                    nc.gpsimd.dma_start(
                        out=wp_sb[c], in_=wproj_d[c * 128:(c + 1) * 128, :])
                bp_row = pp.tile([1, E], F32R)
                nc.gpsimd.dma_start(
                    out=bp_row, in_=bproj_d[:].rearrange("(o f) -> o f", o=1))
                bp_bc = pp.tile([128, E], F32)
                for nf, (f0, fw) in enumerate(NFS):
                    pbp = ps.tile([128, 512], F32, name=f"pbp{nf}", tag="mm",
                                  bufs=2)
                    nc.tensor.matmul(pbp[:, :fw], ones1, bp_row[:, f0:f0 + fw],
                                     start=True, stop=True)
                    nc.vector.tensor_copy(bp_bc[:, f0:f0 + fw], pbp[:, :fw])
                _run_attention_and_proj(
                    nc, tc, ps, mybir, qT, kT, vS, attnT, ones65,
                    wp_sb, bp_bc, out_d)
    nc.compile()
    return nc


def _run_attention_and_proj(nc, tc, ps, mybir, qT, kT, vS, attnT, ones65,
                            wp_sb, bp_bc, out_d):
    import concourse.tile as tile
    F32 = mybir.dt.float32
    F32R = mybir.dt.float32r
    EXP = mybir.ActivationFunctionType.Exp
    tc2 = tc
    if True:
            with tc2.tile_pool(name="exp", bufs=1) as ep:
                for h in range(H):
                    c, r0 = h // 2, (h % 2) * 64
                    expS = [
                        ep.tile([128, N], F32R, name=f"expS{h}_{kc}",
                                tag="expS", bufs=14)
                        for kc in range(NT)]
                    for kc in range(NT):
                        pss = ps.tile([128, N], F32, name=f"ps{h}_{kc}",
                                      tag="s2", bufs=2)
                        for q in range(NQ):
                            nc.tensor.matmul(
                                pss[:, q * 512:(q + 1) * 512],
                                kT[c][r0:r0 + 64, kc * 128:(kc + 1) * 128],
                                qT[c][r0:r0 + 64, q * 512:(q + 1) * 512],
                                start=True, stop=True)
                        nc.scalar.activation(expS[kc], pss, EXP,
                                             scale=float(SCALE))
                    for q in range(NQ):
                        pu = ps.tile([65, 512], F32, name=f"pu{h}_{q}",
                                     tag="u", bufs=2)
                        for kc in range(NT):
                            nc.tensor.matmul(
                                pu,
                                vS[kc][:, h * 65:h * 65 + 65],
                                expS[kc][:, q * 512:(q + 1) * 512],
                                start=(kc == 0), stop=(kc == NT - 1))
                        rz = ep.tile([65, 512], F32R, name=f"rz{h}_{q}",
                                     tag="rz", bufs=2)
                        with nc.allow_low_precision(reason="invZ f32r bcast"):
                            nc.vector.reciprocal(rz[64:65, :], pu[64:65, :])
                        pb = ps.tile([128, 512], F32, name=f"pb{h}_{q}",
                                     tag="mm", bufs=2)
                        nc.tensor.matmul(
                            pb[0:64, :], ones65[64:65, :], rz[64:65, :],
                            start=True, stop=True)
                        pbs = ep.tile([64, 512], F32, name=f"pbs{h}_{q}",
                                      tag="pbs", bufs=2)
                        nc.vector.tensor_copy(pbs, pb[0:64, :])
                        if h % 2 == 0:
                            nc.vector.tensor_mul(
                                attnT[c][0:64, q * 512:(q + 1) * 512],
                                pu[0:64, :], pbs)
                        else:
                            tmp = ep.tile([64, 512], F32R, name=f"tmp{h}_{q}",
                                          tag="tmp", bufs=2)
                            nc.vector.tensor_mul(tmp, pu[0:64, :], pbs)
                            nc.sync.dma_start(
                                out=attnT[c][64:128, q * 512:(q + 1) * 512],
                                in_=tmp)

            # ---- phase 3: output projection (pool pp still open) ----
            with tc2.tile_pool(name="osb", bufs=1) as op:
                for i in range(NT):
                    o_sb = op.tile([128, E], F32, name=f"o{i}", tag="o", bufs=3)
                    for nf, (f0, fw) in enumerate(NFS):
                        po = ps.tile([128, 512], F32, name=f"po{i}_{nf}",
                                     tag=("s2", "mm", "u")[(i * 2 + nf) % 3],
                                     bufs=2)
                        for c in range(6):
                            nc.tensor.matmul(
                                po[:, :fw],
                                attnT[c][:, i * 128:(i + 1) * 128],
                                wp_sb[c][:, f0:f0 + fw],
                                start=(c == 0), stop=(c == 5))
                        nc.vector.tensor_add(
                            o_sb[:, f0:f0 + fw], po[:, :fw],
                            bp_bc[:, f0:f0 + fw])
                    nc.sync.dma_start(
                        out=out_d[i * 128:(i + 1) * 128, :], in_=o_sb)


_NC_CACHE = None


def kernel(x, W_qkv, b_qkv, W_proj, b_proj):
    from concourse.bass_utils import run_bass_kernel_spmd

    global _NC_CACHE
    if _NC_CACHE is None:
        _NC_CACHE = _build()
    nc = _NC_CACHE

    x = np.ascontiguousarray(np.asarray(x, dtype=np.float32))
    W_qkv = np.ascontiguousarray(np.asarray(W_qkv, dtype=np.float32))
    b_qkv = np.ascontiguousarray(np.asarray(b_qkv, dtype=np.float32))
    W_proj = np.ascontiguousarray(np.asarray(W_proj, dtype=np.float32))
    b_proj = np.ascontiguousarray(np.asarray(b_proj, dtype=np.float32))

    in_maps = [
        {"x": x[b], "W_qkv": W_qkv, "b_qkv": b_qkv,
         "W_proj": W_proj, "b_proj": b_proj}
        for b in range(B)
    ]
    res = run_bass_kernel_spmd(nc, in_maps, core_ids=list(range(B)))
    return np.stack([np.asarray(res.results[b]["out"]) for b in range(B)])
